# revision 91
# baseline (speedup 1.0000x reference)
"""Trainium2 Bass kernel for nn_MultiHeadAttention (Q.V^T attention variant).

fp8 DoubleRow design. Reference computation (B=2, S=2048, F=1024, H=16, D=64):
    q = query @ Wq + bq            -> [B,S,H,D]
    v = value @ Wv + bv            -> [B,S,H,D]
    score = einsum(bqhd,bkhd->bhqk)(q, v) / sqrt(D)
    align = softmax(score, -1)
    ctx = einsum(bhqk,bkhd->bqhd)(align, v)
    out = LN(concat([ctx, query], -1) @ Wfc + bfc) [* gamma + beta]

Sharding: 8 cores = 2 batches x 4 query-row chunks of 512 rows. Each core
projects its own 512 value rows into two fp8 layouts (vT_sA/vT_sB: d-split
2x32 score-lhsT layout via host-permuted Wv columns; V_full: key-major 65-col
head blocks with a ones column for the in-matmul softmax denominator),
AllGathers both within its 4-core batch group, projects its own 512 query
rows, then runs attention + fused concat/fc/LayerNorm.

The softmax exp ([128,1024] per (pair, kt)) is the bottleneck; the schedule
splits it across TWO engines and hides everything else under it:
  - ~25 of the 128 exp tiles run on DVE via an int8 e4m3 bit-trick
    (value = bitcast_e4m3(round(score * 8*log2(e)/sqrt(D) + 55.54)), a
    round-to-nearest piecewise-linear exp2 whose ~3% rms error is on par
    with the fp8 quantization the ACT path already incurs); their score
    matmuls land in the fc psum ring so the ACT stream keeps an
    uninterrupted pps double-buffer
  - AllGather A is split (head pairs 0-1 first, in a dedicated staging tile
    and dedicated vT_sA target so the chain has narrow deps) so attention
    starts early; A2 lands in vT_sB so its scatter never carries a
    conservative WAR against running pair-0/1 score reads
  - the V projection runs inside pair 0's key-tile slots; wvn loads before
    qT16 so the B gathers are not starved by the serial DMA device
  - context matmuls run one pair late as 16-instruction DR bursts; softmax
    normalization broadcasts 1/denom via a DRAM-bounce DMA instead of a PE
    matmul, freeing 2 PSUM banks
  - the fc runs as spill sessions in those 2 banks during attention (partial
    sums accumulate into SBUF via DVE adds); the fc bias is folded into the
    first session so only ctx pair 7's matmul remains after the last exp;
    pair 7's kc4-6 sessions sit late (slots 8-15) so they don't contend
    with its DVE-exp tiles in the fc psum ring
  - the pair 2-7 vT projection stores (t=0 halves) run on ACT via
    activation-Identity bias-adds while ACT is idle pre-attention, halving
    the serial DVE store chain that gates the A2 gather; the 2MB qT16 load
    is gated behind the A2 scatter so the serial DMA device serves the A2
    chain first
  - pair 7's softmax normalization hoists both reciprocals and does the
    psum->SBUF broadcast copies on ACT (idle post-exp), shortening the
    tail's serial DVE chain; a dummy Sqrt preloads the ACT table set;
    LayerNorm stats/affines are split across ACT and DVE (squares 3:1
    toward ACT) with the per-row scalar chain batched into [128,4] columns
"""

import numpy as np
import ml_dtypes

import concourse.bass as bass
import concourse.tile as tile
from concourse import bacc, mybir
from concourse.bass_utils import run_bass_kernel_spmd

FP8 = mybir.dt.float8e4
BF16 = mybir.dt.bfloat16
F32 = mybir.dt.float32
I8 = mybir.dt.int8
NP_FP8 = ml_dtypes.float8_e4m3fn
NP_BF16 = ml_dtypes.bfloat16
DR = mybir.MatmulPerfMode.DoubleRow

B, S, F, H, D = 2, 2048, 1024, 16, 64
NCORES = 8
RPC = 512            # query rows per core
CHUNKS = 4           # row chunks per batch (= cores per batch group)
KEYS = S             # 2048 keys per batch
NKT = KEYS // 128    # 16 key tiles
NPAIR = H // 2       # 8 head pairs
EPS = 1e-5

# (pair, kt) exp tiles offloaded to DVE via the int8 e4m3 bit-trick:
# value = bitcast_e4m3(round(score * 8*log2(e)/sqrt(D) + 55.54)), a
# piecewise-linear exp2 whose error (~3% rms) is comparable to the fp8
# quantization the ACT path already incurs
DVE_EXP = ({(p, kt) for p in range(1, 8) for kt in (3, 7, 11, 14)}
           - {(7, 14), (7, 11), (6, 14)}) | {(7, 2), (7, 5), (6, 5)}
EXP_A = 8.0 * 1.4426950408889634 / 8.0   # 8*log2e*inv_sqrt_d
EXP_B = 55.54

A1_J = 1                              # head pairs in the first A gather
A1_ELEMS = 64 * A1_J * 2 * RPC
A2_ELEMS = 64 * (8 - A1_J) * 2 * RPC
B_ELEMS = 128 * 4 * H * 65            # V own 4 keytiles [128, 4, 16, 65]

DEBUG = False
NO_COLL = False
NO_COLL_FREE = False
APPLY_GB = True


def _perm():
    """Weight-column permutation for the d-split score layout.
    Projection psum j covers head pair g=j: partitions 0:64 hold d-slot t=0
    (rows 32*hh+r for head h=2g+hh, d=r), partitions 64:128 hold slot t=1
    (d=32+r). new col 128*j + p <- old col 128*j + 64*hh + 32*t + r with
    hh=(p%64)//32, t=p//64, r=p%32."""
    p = np.empty(F, np.int64)
    for j in range(8):
        for pp in range(128):
            hh, t, r = (pp % 64) // 32, pp // 64, pp % 32
            p[128 * j + pp] = 128 * j + 64 * hh + 32 * t + r
    return p


PERM = _perm()


def _build_kernel():
    nc = bacc.Bacc(
        "TRN2",
        target_bir_lowering=False,
        debug=False,
        enable_asserts=False,
        num_devices=NCORES,
    )

    qT_d = nc.dram_tensor("qT", [F, RPC], FP8, kind="ExternalInput")
    qT16_d = nc.dram_tensor("qT16", [F, RPC], BF16, kind="ExternalInput")
    vT_d = nc.dram_tensor("vT", [F, RPC], FP8, kind="ExternalInput")
    wqp_d = nc.dram_tensor("wqp", [F, F], FP8, kind="ExternalInput")
    wvp_d = nc.dram_tensor("wvp", [F, F], FP8, kind="ExternalInput")
    wvn_d = nc.dram_tensor("wvn", [F, F], FP8, kind="ExternalInput")
    wfc_d = nc.dram_tensor("wfc", [2 * F, F], BF16, kind="ExternalInput")
    bqp_d = nc.dram_tensor("bqp", [128, 8], F32, kind="ExternalInput")
    bvp_d = nc.dram_tensor("bvp", [128, 8], F32, kind="ExternalInput")
    bvr_d = nc.dram_tensor("bvr", [1, F], F32, kind="ExternalInput")
    bfc_d = nc.dram_tensor("bfc", [1, F], BF16, kind="ExternalInput")
    gam_d = nc.dram_tensor("gam", [1, F], BF16, kind="ExternalInput")
    bet_d = nc.dram_tensor("bet", [1, F], BF16, kind="ExternalInput")
    out_d = nc.dram_tensor("out", [RPC, F], F32, kind="ExternalOutput")
    dbg = None
    if DEBUG:
        dbg = {
            "dbg_qTs": nc.dram_tensor("dbg_qTs", [64, 16 * RPC], FP8,
                                      kind="ExternalOutput"),
            "dbg_vTs": nc.dram_tensor("dbg_vTs", [64, 16 * KEYS], FP8,
                                      kind="ExternalOutput"),
            "dbg_V": nc.dram_tensor("dbg_V", [128, NKT * H * 65], FP8,
                                    kind="ExternalOutput"),
            "dbg_pt": nc.dram_tensor("dbg_pt", [128, NKT * 1024], FP8,
                                     kind="ExternalOutput"),
            "dbg_ctx": nc.dram_tensor("dbg_ctx", [128, NPAIR * RPC], BF16,
                                      kind="ExternalOutput"),
        }

    with tile.TileContext(nc) as tc:
        _kernel_body(tc, qT_d, qT16_d, vT_d, wqp_d, wvp_d, wvn_d, wfc_d,
                     bqp_d, bvp_d, bvr_d, bfc_d, gam_d, bet_d, out_d, dbg)

    nc.compile()
    return nc


def _stub_src(t, n):
    # [1, n] read of a tile's first row (dep-stub source)
    idx = (slice(0, 1),) + (0,) * (len(t.shape) - 2) + (slice(0, n),)
    return t[idx]


def _bcast_ap(t, nparts, n, row=0):
    # t: dram_tensor handle (has .ap() method) or an AP (dram pool tile)
    base = t.ap() if callable(getattr(t, "ap", None)) else t
    return bass.AP(tensor=base.tensor, offset=base.offset + row * n,
                   ap=[[0, nparts], [1, n]])


def _kernel_body(tc, qT_d, qT16_d, vT_d, wqp_d, wvp_d, wvn_d, wfc_d,
                 bqp_d, bvp_d, bvr_d, bfc_d, gam_d, bet_d, out_d, dbg=None):
    nc = tc.nc
    Exp = mybir.ActivationFunctionType.Exp
    Sqrt = mybir.ActivationFunctionType.Sqrt
    Ident = mybir.ActivationFunctionType.Identity
    Square = mybir.ActivationFunctionType.Square
    mult = mybir.AluOpType.mult
    addop = mybir.AluOpType.add

    import contextlib
    ctx = contextlib.ExitStack()
    with ctx:
        persist = ctx.enter_context(tc.tile_pool(name="persist", bufs=1))
        ptpool = ctx.enter_context(tc.tile_pool(name="ptpool", bufs=2))
        lnp = ctx.enter_context(tc.tile_pool(name="lnp", bufs=2))
        fcpool = ctx.enter_context(tc.tile_pool(name="fcpool", bufs=4))
        wblk = ctx.enter_context(tc.tile_pool(name="wblk", bufs=8))
        small = ctx.enter_context(tc.tile_pool(name="small", bufs=2))
        bcpool = ctx.enter_context(tc.tile_pool(name="bcpool", bufs=2))
        pps = ctx.enter_context(tc.tile_pool(name="pps", bufs=2, space="PSUM"))
        pctx = ctx.enter_context(tc.tile_pool(name="pctx", bufs=1, space="PSUM"))
        pfc = ctx.enter_context(tc.tile_pool(name="pfc", bufs=2, space="PSUM"))
        dram = ctx.enter_context(tc.tile_pool(name="dram", bufs=1, space="DRAM"))

        # ---- persistent SBUF ----
        qTin = persist.tile([128, 8, RPC], FP8)       # query chunk, dtile-major
        vTin = persist.tile([128, 8, RPC], FP8)       # value chunk
        qT_s = persist.tile([64, 8, 2, RPC], FP8)     # projected q, d-split layout
        # separate A1/A2 gather targets and staging: the A1 chain must not
        # wait on later pairs' projection stores, and the A2 scatter must not
        # carry a conservative WAR against pair-0/1 score reads
        vT_sA = persist.tile([64, A1_J, 2, KEYS], FP8)
        vT_sB = persist.tile([64, 8 - A1_J, 2, KEYS], FP8)
        V_full = persist.tile([128, NKT, H, 65], FP8) # projected V, 65-col head blocks
        vTstageA = persist.tile([64, A1_J, 2, RPC], FP8)
        vTstage = persist.tile([64, 8 - A1_J, 2, RPC], FP8)
        Vstage = persist.tile([128, 4, H, 65], FP8)   # own V keytiles (AG payload B)
        ctxT = persist.tile([128, NPAIR, RPC], BF16)  # normalized context^T
        qT16 = persist.tile([128, 8, RPC], BF16)      # query chunk bf16 (fc lhsT)
        fcq = persist.tile([128, 8, 512], F32)        # fc partial sums (qb, c)
        ones_bf = persist.tile([1, 512], BF16)
        ones64f = persist.tile([1, 64], BF16)
        bq_sb = persist.tile([128, 8], F32)
        bv_sb = persist.tile([128, 8], F32)
        bvr_bc = persist.tile([128, F], F32)
        bfc_sb = persist.tile([1, F], BF16)
        eps_sb = persist.tile([128, 1], F32)
        ssum_all = persist.tile([128, 8], F32)
        sqsum_all = persist.tile([128, 8], F32)
        if APPLY_GB:
            gamma_bc = persist.tile([128, F], BF16)
            beta_bc = persist.tile([128, F], BF16)

        ag_inA1 = dram.tile([A1_ELEMS], FP8)
        ag_outA1 = dram.tile([CHUNKS, A1_ELEMS], FP8)
        ag_inA2 = dram.tile([A2_ELEMS], FP8)
        ag_outA2 = dram.tile([CHUNKS, A2_ELEMS], FP8)
        ag_inB1 = dram.tile([B_ELEMS // 2], FP8)
        ag_outB1 = dram.tile([CHUNKS, B_ELEMS // 2], FP8)
        ag_inB2 = dram.tile([B_ELEMS // 2], FP8)
        ag_outB2 = dram.tile([CHUNKS, B_ELEMS // 2], FP8)
        rec_d = dram.tile([H, RPC], BF16)             # normalize recip bounce

        nc.vector.memset(ones_bf[:, :], 1.0)
        nc.vector.memset(ones64f[:, :], 1.0)
        nc.vector.memset(eps_sb[:, :], EPS)
        nc.vector.memset(Vstage[:, :, :, 64:65], 1.0)

        # ---- input DMAs: the vT path first (it feeds the A1 gather chain),
        # then the q path ----
        wvp_blks = []
        wqp_blks = []
        for kp in range(4):
            wb = wblk.tile([128, 2, F], FP8, tag="wblk", name=f"wvp{kp}")
            nc.sync.dma_start(
                out=wb[:, :, :],
                in_=wvp_d[256 * kp:256 * (kp + 1), :].rearrange(
                    "(t p) n -> p t n", p=128))
            wvp_blks.append(wb)
            nc.sync.dma_start(
                out=vTin[:, 2 * kp:2 * kp + 2, :],
                in_=vT_d[256 * kp:256 * (kp + 1), :].rearrange(
                    "(m p) n -> p m n", p=128))
        nc.sync.dma_start(out=bv_sb[:, :], in_=bvp_d[:, :])
        for kp in range(4):
            wq = wblk.tile([128, 2, F], FP8, tag="wblk", name=f"wqp{kp}")
            nc.sync.dma_start(
                out=wq[:, :, :],
                in_=wqp_d[256 * kp:256 * (kp + 1), :].rearrange(
                    "(t p) n -> p t n", p=128))
            wqp_blks.append(wq)
            nc.sync.dma_start(
                out=qTin[:, 2 * kp:2 * kp + 2, :],
                in_=qT_d[256 * kp:256 * (kp + 1), :].rearrange(
                    "(m p) n -> p m n", p=128))
        nc.sync.dma_start(out=bq_sb[:, :], in_=bqp_d[:, :])

        def all_gather(in_ap, out_ap):
            if NO_COLL_FREE:
                nc.sync.dma_start(out=out_ap[0], in_=in_ap)
            elif NO_COLL:
                n = in_ap.free_size() * in_ap.partition_size() \
                    if hasattr(in_ap, 'partition_size') else None
                src_bc = bass.AP(tensor=in_ap.tensor, offset=in_ap.offset,
                                 ap=[[0, CHUNKS]] + list(in_ap.ap))
                nc.sync.dma_start(out=out_ap[:, :], in_=src_bc)
            else:
                nc.gpsimd.collective_compute(
                    "AllGather",
                    mybir.AluOpType.bypass,
                    replica_groups=[[0, 1, 2, 3], [4, 5, 6, 7]],
                    ins=[in_ap],
                    outs=[out_ap],
                )

        def gather_chain_A(agin, agout, srctile, npairs, dsttile):
            nj = 2 * npairs
            nc.sync.dma_start(
                out=agin[:].rearrange("(j p n) -> p j n", p=64, j=nj),
                in_=srctile[0:64, :, :, :].rearrange("p g t n -> p (g t) n"))
            all_gather(agin[:], agout[:, :])
            for r in range(CHUNKS):
                nc.sync.dma_start(
                    out=dsttile[0:64, :, :, :].rearrange(
                        "p g t n -> p (g t) n")[:, :, r * RPC:(r + 1) * RPC],
                    in_=agout[r, :].rearrange("(j p n) -> p j n", p=64, j=nj))

        def proj_j(blks, xin, dst, dstj, bias_sb, ps_j, j, act_store=False):
            for kp in range(4):
                nc.tensor.matmul(ps_j[:, :],
                                 blks[kp][:, :, 128 * j:128 * (j + 1)],
                                 xin[:, 2 * kp:2 * kp + 2, :],
                                 start=(kp == 0), stop=(kp == 3),
                                 perf_mode=DR)
            with nc.allow_low_precision(reason="fp8 proj store"):
                if act_store:
                    # ACT is idle pre-attention; bias-add store via
                    # activation Identity shortens the serial DVE store
                    # chain that gates the A2 stage
                    nc.scalar.activation(
                        dst[0:64, dstj, 0, :], ps_j[0:64, :],
                        mybir.ActivationFunctionType.Identity,
                        bias=bias_sb[0:64, j:j + 1])
                else:
                    nc.vector.tensor_scalar(
                        dst[0:64, dstj, 0, :], ps_j[0:64, :],
                        bias_sb[0:64, j:j + 1], None, op0=addop)
                nc.vector.tensor_scalar(
                    dst[0:64, dstj, 1, :], ps_j[64:128, :],
                    bias_sb[64:128, j:j + 1], None, op0=addop)

        # ---- pre-attention: vT_s pairs 0,1 -> gather A1 -> q pairs 0,1 ----
        pre = []
        for i in range(2):
            big = pps.tile([128, 1024], F32, tag="ps", name=f"pre{i}")
            pre.append(big[:, 0:512])
            pre.append(big[:, 512:1024])
        for j in range(A1_J):
            proj_j(wvp_blks, vTin, vTstageA, j, bv_sb, pre[j], j,
                   act_store=True)
        gather_chain_A(ag_inA1, ag_outA1, vTstageA, A1_J, vT_sA)
        for j in range(2):
            proj_j(wqp_blks, qTin, qT_s, j, bq_sb, pre[2 + j], j)

        # ---- bulk DMAs deferred behind the A1 chain: tiny ACT copies
        # reading vT_s (written by the A1 scatters) gate them via WAW on
        # their destination tiles, keeping the DMA device clear while the
        # first scores' operands land ----
        Copy = mybir.ActivationFunctionType.Copy
        nc.scalar.activation(bvr_bc[0:1, 0:1], _stub_src(vT_sA, 1), Copy)
        nc.sync.dma_start(out=bvr_bc[:, :], in_=_bcast_ap(bvr_d, 128, F))
        wvn_blks = []
        for kp in range(4):
            wb = wblk.tile([128, 2, F], FP8, tag="wblk", name=f"wvn{kp}")
            nc.scalar.activation(wb[0:1, 0, 0:1], _stub_src(vT_sA, 1), Copy)
            nc.sync.dma_start(
                out=wb[:, :, :],
                in_=wvn_d[256 * kp:256 * (kp + 1), :].rearrange(
                    "(t p) n -> p t n", p=128))
            wvn_blks.append(wb)
        nc.scalar.activation(qT16[0:1, :, 0:1], _stub_src(vT_sB, 8), Copy)
        for m in range(8):
            nc.sync.dma_start(
                out=qT16[:, m, :],
                in_=qT16_d[128 * m:128 * (m + 1), :])
        wfc_blks = [None] * 8

        # ---- attention-phase work units (hooked into the kt loop) ----
        inv_sqrt_d = 1.0 / np.sqrt(D)
        pt_tiles = {}
        cps_tiles = {}

        def hook_psum(name):
            # rotate the 4 non-score psum tiles for hook-phase matmul groups
            i = hook_psum.n % 4
            hook_psum.n += 1
            if i < 2:
                return pctx.tile([128, RPC], F32, tag=("ctxA", "ctxB")[i],
                                 name=name)
            return pfc.tile([128, RPC], F32, tag="fc", name=name)
        hook_psum.n = 0

        def proj_hook(blks, xin, dst, dstj, bias_sb, j, act_store=False):
            proj_j(blks, xin, dst, dstj, bias_sb, hook_psum(f"pj{j}"), j,
                   act_store=act_store)

        def v_group(l, c):
            # V projection group: own keytile l, column half c
            ps_ = hook_psum(f"vg{l}{c}")
            for kp in range(4):
                nc.tensor.matmul(ps_[:, :],
                                 vTin[:, 2 * kp:2 * kp + 2,
                                      128 * l:128 * (l + 1)],
                                 wvn_blks[kp][:, :, 512 * c:512 * (c + 1)],
                                 start=(kp == 0), stop=(kp == 3),
                                 perf_mode=DR)
            with nc.allow_low_precision(reason="fp8 V store"):
                nc.vector.scalar_tensor_tensor(
                    Vstage[:, l, 8 * c:8 * (c + 1), 0:64],
                    ps_[:, :].rearrange("p (h d) -> p h d", d=64),
                    1.0,
                    bvr_bc[:, 512 * c:512 * (c + 1)].rearrange(
                        "p (h d) -> p h d", d=64),
                    op0=mult, op1=addop)

        def gather_chain_B(ls, agin, agout):
            # gather half the V keytiles (ls = (0,1) or (2,3))
            nl = len(ls)
            nc.sync.dma_start(
                out=agin[:].rearrange("(l p n) -> p l n", p=128, l=nl),
                in_=Vstage[:, ls[0]:ls[0] + nl, :, :].rearrange(
                    "p l h e -> p l (h e)"))
            all_gather(agin[:], agout[:, :])
            for r in range(CHUNKS):
                nc.sync.dma_start(
                    out=V_full[:, 4 * r + ls[0]:4 * r + ls[0] + nl, :, :]
                        .rearrange("p l h e -> p l (h e)"),
                    in_=agout[r, :].rearrange("(l p n) -> p l n", p=128, l=nl))

        def ctx_part(pp, j2s, first, last):
            if first:
                cps_tiles[pp] = (
                    pctx.tile([65, RPC], F32, tag="ctxA", name=f"cA{pp}"),
                    pctx.tile([65, RPC], F32, tag="ctxB", name=f"cB{pp}"))
            cpsA, cpsB = cps_tiles[pp]
            pt = pt_tiles[pp]
            for n, j2 in enumerate(j2s):
                for hh, cps in ((0, cpsA), (1, cpsB)):
                    nc.tensor.matmul(
                        cps[:, :],
                        V_full[:, 2 * j2:2 * j2 + 2, 2 * pp + hh, :],
                        pt[:, 2 * j2:2 * j2 + 2, hh, :],
                        start=(first and n == 0),
                        stop=(last and n == len(j2s) - 1),
                        perf_mode=DR)

        def ctx_burst(pp):
            # B1 keytiles (4r, 4r+1 -> even j2) first, then B2 (odd j2)
            ctx_part(pp, [0, 2, 4, 6, 1, 3, 5, 7], True, True)

        def normalize_pair(pp, pe_bc=False):
            cpsA, cpsB = cps_tiles[pp]
            if pe_bc:
                # tail path: hoist both recips, do the psum->SBUF broadcast
                # copies on ACT (idle after the last exp) so DVE only runs
                # recipA, recipB, multA, multB on the critical chain
                Copy_ = mybir.ActivationFunctionType.Copy
                recs, bcss = [], []
                for hh, cps in ((0, cpsA), (1, cpsB)):
                    rec = small.tile([1, RPC], BF16, tag="rec", name="rec")
                    with nc.allow_low_precision(
                            reason="softmax denom recip bf16"):
                        nc.vector.reciprocal(rec[:, :], cps[64:65, :])
                    recs.append(rec)
                for hh, cps in ((0, cpsA), (1, cpsB)):
                    bcs = bcpool.tile([64, RPC], BF16, tag="bcs", name="bcs")
                    bcp = pfc.tile([64, RPC], F32, tag="fc", name="bcp")
                    nc.tensor.matmul(bcp[:, :], ones64f[:, :],
                                     recs[hh][:, :], start=True, stop=True)
                    with nc.allow_low_precision(reason="bcs copy bf16"):
                        nc.scalar.activation(bcs[:, :], bcp[:, :], Copy_)
                    bcss.append(bcs)
                for hh, cps in ((0, cpsA), (1, cpsB)):
                    nc.vector.tensor_tensor(
                        ctxT[64 * hh:64 * (hh + 1), pp, :],
                        cps[0:64, :], bcss[hh][:, :], op=mult)
                return
            for hh, cps in ((0, cpsA), (1, cpsB)):
                rec = small.tile([1, RPC], BF16, tag="rec", name="rec")
                with nc.allow_low_precision(reason="softmax denom recip bf16"):
                    nc.vector.reciprocal(rec[:, :], cps[64:65, :])
                bcs = bcpool.tile([64, RPC], BF16, tag="bcs", name="bcs")
                nc.sync.dma_start(out=rec_d[2 * pp + hh, :], in_=rec[:, :])
                nc.sync.dma_start(
                    out=bcs[:, :],
                    in_=_bcast_ap(rec_d, 64, RPC, row=2 * pp + hh))
                nc.vector.tensor_tensor(
                    ctxT[64 * hh:64 * (hh + 1), pp, :],
                    cps[0:64, :], bcs[:, :], op=mult)

        def fc_session(i, kcs, first=False, last=False):
            qb, c = i // 2, i % 2
            ps_ = pfc.tile([128, RPC], F32, tag="fc", name=f"s{i}")
            kcs = list(kcs)
            for n, kc in enumerate(kcs):
                if kc < 8:
                    lhsT = ctxT[:, kc, 128 * qb:128 * (qb + 1)]
                else:
                    lhsT = qT16[:, kc - 8, 128 * qb:128 * (qb + 1)]
                nc.tensor.matmul(
                    ps_[:, :], lhsT,
                    wfc_blks[kc // 2][:, kc % 2, 512 * c:512 * (c + 1)],
                    start=(n == 0),
                    stop=(not first and n == len(kcs) - 1))
            if first:
                # fold the fc bias into the first session
                nc.tensor.matmul(ps_[:, :], ones_bf[:, 0:128],
                                 bfc_sb[:, 512 * c:512 * (c + 1)],
                                 start=False, stop=True)
                nc.vector.tensor_copy(fcq[:, i, :], ps_[:, :])
            elif last:
                nc.vector.scalar_tensor_tensor(
                    fcq[:, i, :], ps_[:, :], 1.0, fcq[:, i, :],
                    op0=mult, op1=addop, accum_out=ssum_all[:, i:i + 1])
            else:
                nc.vector.tensor_tensor(fcq[:, i, :], ps_[:, :], fcq[:, i, :],
                                        op=addop)

        # hook schedule: (pair, kt) -> list of work closures
        hooks = {}

        def add_hook(p, kt, fn):
            hooks.setdefault((p, kt), []).append(fn)

        # pair 0: finish vT pairs 1-7, gather A2, V groups + split B gathers
        for j in range(A1_J, 8):
            add_hook(0, j - A1_J, lambda j=j: proj_hook(wvp_blks, vTin,
                                                        vTstage, j - A1_J,
                                                        bv_sb, j,
                                                        act_store=True))
        add_hook(0, 6, lambda: gather_chain_A(ag_inA2, ag_outA2, vTstage,
                                              8 - A1_J, vT_sB))
        for g in range(4):
            add_hook(0, 8 + g, lambda g=g: v_group(g // 2, g % 2))
        add_hook(0, 11, lambda: gather_chain_B((0, 1), ag_inB1, ag_outB1))
        for g in range(4, 8):
            add_hook(0, 8 + g, lambda g=g: v_group(g // 2, g % 2))
        def late_dmas():
            with nc.allow_low_precision(reason="dep stub"):
                nc.gpsimd.tensor_copy(bfc_sb[0:1, 0:1].bitcast(FP8)[:, 0:1],
                                      V_full[0:1, 13, 0, 0:1])
            nc.sync.dma_start(out=bfc_sb[:, :], in_=bfc_d[:, :])
            if APPLY_GB:
                nc.sync.dma_start(out=gamma_bc[:, :],
                                  in_=_bcast_ap(gam_d, 128, F))
                nc.sync.dma_start(out=beta_bc[:, :],
                                  in_=_bcast_ap(bet_d, 128, F))
            for jj in (4, 5, 6, 7, 0, 1, 2, 3):
                wb = wblk.tile([128, 2, F], BF16, tag="wblk", name=f"wfc{jj}")
                nc.sync.dma_start(
                    out=wb[:, :, :],
                    in_=wfc_d[256 * jj:256 * (jj + 1), :].rearrange(
                        "(t p) n -> p t n", p=128))
                wfc_blks[jj] = wb

        # pair 1: B2 gather, late bulk DMAs, q pairs 2-7
        add_hook(1, 0, lambda: gather_chain_B((2, 3), ag_inB2, ag_outB2))
        add_hook(1, 1, late_dmas)
        for j in range(2, 8):
            add_hook(1, j - 1, lambda j=j: proj_hook(wqp_blks, qTin, qT_s, j,
                                                     bq_sb, j))
        if dbg is not None:
            def dump_pt0():
                nc.sync.dma_start(
                    out=dbg["dbg_pt"][:, :],
                    in_=pt_tiles[0][:, :, :, :].rearrange("p k h n -> p (k h n)"))
            add_hook(1, 7, dump_pt0)
        # pair 1 tail/pair 2: ctx/norm pair 0, ctx pair 1
        add_hook(1, 12, lambda: ctx_burst(0))
        add_hook(2, 4, lambda: normalize_pair(0))
        add_hook(2, 8, lambda: ctx_burst(1))
        # pair 3: norm 1, ctx/norm 2, fc query sessions 0-3
        add_hook(2, 15, lambda: normalize_pair(1))
        for i in range(4):
            add_hook(3, 2 + 4 * i, lambda i=i: fc_session(i, range(8, 16),
                                                          first=True))
        add_hook(3, 6, lambda: ctx_burst(2))
        add_hook(3, 12, lambda: normalize_pair(2))
        # pair 4: ctx/norm 3, fc query sessions 4-7
        for i in range(4):
            add_hook(4, 2 + 4 * i, lambda i=i: fc_session(4 + i, range(8, 16),
                                                          first=True))
        add_hook(4, 4, lambda: ctx_burst(3))
        add_hook(4, 12, lambda: normalize_pair(3))
        for p in (5, 6):
            add_hook(p, 2, lambda pp=p - 1: ctx_burst(pp))
            add_hook(p, 10, lambda pp=p - 1: normalize_pair(pp))
        for i in range(4):
            add_hook(5, 2 + 4 * i, lambda i=i: fc_session(i, range(0, 4)))
            add_hook(6, 2 + 4 * i, lambda i=i: fc_session(4 + i, range(0, 4)))
        # pair 7: ctx/norm 6 early, fc kc 4-6 sessions, ctx 7 nearly inline
        add_hook(7, 0, lambda: ctx_burst(6))
        add_hook(7, 2, lambda: normalize_pair(6))
        for i in range(8):
            add_hook(7, 8 + i, lambda i=i: fc_session(i, range(4, 7)))
        add_hook(7, 14, lambda: ctx_part(7, [0, 2, 4, 1, 3, 5, 6], True, False))

        # ---- attention ----
        for p in range(NPAIR):
            pt = ptpool.tile([128, NKT, 2, RPC], FP8, tag="pt", name="pt")
            pt_tiles[p] = pt
            vts, vj = (vT_sA, p) if p < A1_J else (vT_sB, p - A1_J)
            for kt in range(NKT):
                if (p, kt) in DVE_EXP:
                    # DVE-exp tiles use the fc psum ring so the ACT stream's
                    # pps ring stays an uninterrupted double-buffer
                    with nc.allow_low_precision(reason="dve int8 exp trick"):
                        for hh in range(2):
                            psd = pfc.tile([128, RPC], F32, tag="fc",
                                           name=f"dve{hh}")
                            nc.tensor.matmul(
                                psd[:, :],
                                vts[32 * hh:32 * (hh + 1), vj, :,
                                    128 * kt:128 * (kt + 1)],
                                qT_s[32 * hh:32 * (hh + 1), p, :, :],
                                start=True, stop=True, perf_mode=DR)
                            nc.vector.tensor_scalar(
                                pt[:, kt, hh, :].bitcast(I8), psd[:, :],
                                EXP_A, EXP_B, op0=mult, op1=addop)
                else:
                    ps = pps.tile([128, 1024], F32, tag="ps", name="ps")
                    for hh in range(2):
                        nc.tensor.matmul(
                            ps[:, 512 * hh:512 * (hh + 1)],
                            vts[32 * hh:32 * (hh + 1), vj, :,
                                128 * kt:128 * (kt + 1)],
                            qT_s[32 * hh:32 * (hh + 1), p, :, :],
                            start=True, stop=True, perf_mode=DR)
                    nc.scalar.activation(pt[:, kt, :, :], ps[:, :], Exp,
                                         scale=inv_sqrt_d)
                for fn in hooks.get((p, kt), ()):
                    fn()

        # ---- tail: finish ctx 7, normalize (PE broadcast), final fc, LN ----
        # dummy Sqrt preloads the sqrt table set while ACT is otherwise idle
        dmy = small.tile([1, 1], F32, tag="dmy", name="dmy")
        nc.scalar.activation(dmy[0:1, 0:1], eps_sb[0:1, 0:1], Sqrt)
        ctx_part(7, [7], False, True)
        normalize_pair(7, pe_bc=True)
        if dbg is not None:
            nc.sync.dma_start(out=dbg["dbg_qTs"][:, :],
                              in_=qT_s[:, :, :, :].rearrange("p g t n -> p (g t n)"))
            nc.sync.dma_start(out=dbg["dbg_vTs"][:, :],
                              in_=vT_sB[:, :, :, :].rearrange("p g t n -> p (g t n)"))
            nc.sync.dma_start(out=dbg["dbg_V"][:, :],
                              in_=V_full[:, :, :, :].rearrange("p k h e -> p (k h e)"))
            nc.sync.dma_start(out=dbg["dbg_ctx"][:, :],
                              in_=ctxT[:, :, :].rearrange("p q n -> p (q n)"))
        for i in range(8):
            fc_session(i, range(7, 8), last=True)

        # LayerNorm: sq-stats on ACT (sum came free from the fc adds), the
        # per-row scalar chain batched into [128, 4] columns, affines split
        # across ACT/DVE.
        for i in range(8):
            dump = lnp.tile([128, 512], F32, tag=f"sq{i % 4}", name="dump",
                            bufs=1)
            eng = (nc.scalar, nc.scalar, nc.scalar, nc.vector)[i % 4]
            if eng is nc.scalar:
                nc.scalar.activation(dump[:, :], fcq[:, i, :], Square,
                                     accum_out=sqsum_all[:, i:i + 1])
            else:
                eng.scalar_tensor_tensor(
                    dump[:, :], fcq[:, i, :], 1.0, fcq[:, i, :],
                    op0=mult, op1=mult, accum_out=sqsum_all[:, i:i + 1])
        mean = small.tile([128, 4], F32, tag="mean", name="mean")
        nc.vector.tensor_tensor(mean[:, :], ssum_all[:, 0::2],
                                ssum_all[:, 1::2], op=addop)
        nc.vector.tensor_scalar(mean[:, :], mean[:, :], 1.0 / F, None, op0=mult)
        ex2 = small.tile([128, 4], F32, tag="ex2", name="ex2")
        nc.vector.tensor_tensor(ex2[:, :], sqsum_all[:, 0::2],
                                sqsum_all[:, 1::2], op=addop)
        nc.vector.tensor_scalar(ex2[:, :], ex2[:, :], 1.0 / F, None, op0=mult)
        var = small.tile([128, 4], F32, tag="var", name="var")
        nc.vector.scalar_tensor_tensor(var[:, :], mean[:, :], -1.0, mean[:, :],
                                       op0=mult, op1=mult)
        nc.vector.tensor_tensor(var[:, :], ex2[:, :], var[:, :], op=addop)
        sd = small.tile([128, 4], F32, tag="sd", name="sd")
        nc.scalar.activation(sd[:, :], var[:, :], Sqrt, bias=eps_sb[:, :])
        rstd = small.tile([128, 4], F32, tag="rstd", name="rstd")
        nc.vector.reciprocal(rstd[:, :], sd[:, :])
        nmr = small.tile([128, 4], F32, tag="nmr", name="nmr")
        nc.vector.scalar_tensor_tensor(nmr[:, :], mean[:, :], -1.0, rstd[:, :],
                                       op0=mult, op1=mult)
        for i in range(8):
            qb, c = i // 2, i % 2
            sl = slice(512 * c, 512 * (c + 1))
            outt = fcpool.tile([128, 512], F32, tag="outt", name="outt")
            if i % 2 == 0:
                nc.scalar.activation(outt[:, :], fcq[:, i, :], Ident,
                                     bias=nmr[:, qb:qb + 1],
                                     scale=rstd[:, qb:qb + 1])
            else:
                nc.vector.tensor_scalar(outt[:, :], fcq[:, i, :],
                                        rstd[:, qb:qb + 1], nmr[:, qb:qb + 1],
                                        op0=mult, op1=addop)
            if APPLY_GB:
                t2 = lnp.tile([128, 512], F32, tag="t1", name="t2",
                              bufs=1)
                nc.vector.tensor_tensor(t2[:, :], outt[:, :],
                                        gamma_bc[:, sl], op=mult)
                nc.vector.tensor_tensor(outt[:, :], t2[:, :],
                                        beta_bc[:, sl], op=addop)
            nc.sync.dma_start(out=out_d[128 * qb:128 * (qb + 1), sl],
                              in_=outt[:, :])


_NC_CACHE = {}


def _get_nc():
    key = (APPLY_GB, NO_COLL, DEBUG)
    if key not in _NC_CACHE:
        _NC_CACHE[key] = _build_kernel()
    return _NC_CACHE[key]


def _prep_inputs(query, value, Wq, bq, Wv, bv, Wfc, bfc, gamma, beta):
    wqp = np.ascontiguousarray(Wq[:, PERM]).astype(NP_FP8)
    wvp = np.ascontiguousarray(Wv[:, PERM]).astype(NP_FP8)
    wvn = np.ascontiguousarray(Wv).astype(NP_FP8)
    wfc16 = np.ascontiguousarray(Wfc).astype(NP_BF16)
    bqp = np.ascontiguousarray(bq[PERM].reshape(8, 128).T).astype(np.float32)
    bvp = np.ascontiguousarray(bv[PERM].reshape(8, 128).T).astype(np.float32)
    bvr = np.ascontiguousarray(bv[None, :]).astype(np.float32)
    bfc16 = np.ascontiguousarray(bfc[None, :]).astype(NP_BF16)
    gam = np.ascontiguousarray(gamma[None, :]).astype(NP_BF16)
    bet = np.ascontiguousarray(beta[None, :]).astype(NP_BF16)

    in_maps = []
    for c in range(NCORES):
        b, r = c // CHUNKS, (c % CHUNKS) * RPC
        qTf = np.ascontiguousarray(query[b, r:r + RPC, :].T)
        qT = qTf.astype(NP_FP8)
        qT16 = qTf.astype(NP_BF16)
        vT = np.ascontiguousarray(value[b, r:r + RPC, :].T).astype(NP_FP8)
        in_maps.append({
            "qT": qT, "qT16": qT16, "vT": vT,
            "wqp": wqp, "wvp": wvp, "wvn": wvn, "wfc": wfc16,
            "bqp": bqp, "bvp": bvp, "bvr": bvr, "bfc": bfc16,
            "gam": gam, "bet": bet,
        })
    return in_maps


def run_on_hw(in_maps, **kwargs):
    nc = _get_nc()
    return run_bass_kernel_spmd(nc, in_maps, list(range(NCORES)), **kwargs)


def kernel(query, value, Wq, bq, Wv, bv, Wfc, bfc, gamma, beta):
    global APPLY_GB
    APPLY_GB = not (np.all(np.asarray(gamma, np.float32) == 1.0)
                    and np.all(np.asarray(beta, np.float32) == 0.0))
    query = np.asarray(query, dtype=np.float32)
    value = np.asarray(value, dtype=np.float32)
    in_maps = _prep_inputs(query, value,
                           np.asarray(Wq, np.float32), np.asarray(bq, np.float32),
                           np.asarray(Wv, np.float32), np.asarray(bv, np.float32),
                           np.asarray(Wfc, np.float32), np.asarray(bfc, np.float32),
                           np.asarray(gamma, np.float32), np.asarray(beta, np.float32))
    res = run_on_hw(in_maps)
    out = np.empty((B, S, F), np.float32)
    for c in range(NCORES):
        b, r = c // CHUNKS, (c % CHUNKS) * RPC
        out[b, r:r + RPC, :] = res.results[c]["out"]
    return out



# revision 94
# speedup vs baseline: 1.0066x; 1.0066x over previous
"""Trainium2 Bass kernel for nn_MultiHeadAttention (Q.V^T attention variant).

fp8 DoubleRow design. Reference computation (B=2, S=2048, F=1024, H=16, D=64):
    q = query @ Wq + bq            -> [B,S,H,D]
    v = value @ Wv + bv            -> [B,S,H,D]
    score = einsum(bqhd,bkhd->bhqk)(q, v) / sqrt(D)
    align = softmax(score, -1)
    ctx = einsum(bhqk,bkhd->bqhd)(align, v)
    out = LN(concat([ctx, query], -1) @ Wfc + bfc) [* gamma + beta]

Sharding: 8 cores = 2 batches x 4 query-row chunks of 512 rows. Each core
projects its own 512 value rows into two fp8 layouts (vT_sA/vT_sB: d-split
2x32 score-lhsT layout via host-permuted Wv columns; V_full: key-major 65-col
head blocks with a ones column for the in-matmul softmax denominator),
AllGathers both within its 4-core batch group, projects its own 512 query
rows, then runs attention + fused concat/fc/LayerNorm.

The softmax exp ([128,1024] per (pair, kt)) is the bottleneck; the schedule
splits it across TWO engines and hides everything else under it:
  - ~25 of the 128 exp tiles run on DVE via an int8 e4m3 bit-trick
    (value = bitcast_e4m3(round(score * 8*log2(e)/sqrt(D) + 55.54)), a
    round-to-nearest piecewise-linear exp2 whose ~3% rms error is on par
    with the fp8 quantization the ACT path already incurs); their score
    matmuls land in the fc psum ring so the ACT stream keeps an
    uninterrupted pps double-buffer
  - AllGather A is split (head pairs 0-1 first, in a dedicated staging tile
    and dedicated vT_sA target so the chain has narrow deps) so attention
    starts early; A2 lands in vT_sB so its scatter never carries a
    conservative WAR against running pair-0/1 score reads
  - the V projection runs inside pair 0's key-tile slots; wvn loads before
    qT16 so the B gathers are not starved by the serial DMA device
  - context matmuls run one pair late as 16-instruction DR bursts; softmax
    normalization broadcasts 1/denom via a DRAM-bounce DMA instead of a PE
    matmul, freeing 2 PSUM banks
  - the fc runs as spill sessions in those 2 banks during attention (partial
    sums accumulate into SBUF via DVE adds); the fc bias is folded into the
    first session so only ctx pair 7's matmul remains after the last exp;
    pair 7's kc4-6 sessions sit late (slots 8-15) so they don't contend
    with its DVE-exp tiles in the fc psum ring
  - the pair 2-7 vT projection stores (t=0 halves) run on ACT via
    activation-Identity bias-adds while ACT is idle pre-attention, halving
    the serial DVE store chain that gates the A2 gather; the 2MB qT16 load
    is gated behind the A2 scatter so the serial DMA device serves the A2
    chain first
  - pair 7's softmax normalization hoists both reciprocals and does the
    psum->SBUF broadcast copies on ACT (idle post-exp), shortening the
    tail's serial DVE chain; a dummy Sqrt preloads the ACT table set;
    LayerNorm stats/affines are split across ACT and DVE (squares 3:1
    toward ACT) with the per-row scalar chain batched into [128,4] columns
"""

import numpy as np
import ml_dtypes

import concourse.bass as bass
import concourse.tile as tile
from concourse import bacc, mybir
from concourse.bass_utils import run_bass_kernel_spmd

FP8 = mybir.dt.float8e4
BF16 = mybir.dt.bfloat16
F32 = mybir.dt.float32
I8 = mybir.dt.int8
NP_FP8 = ml_dtypes.float8_e4m3fn
NP_BF16 = ml_dtypes.bfloat16
DR = mybir.MatmulPerfMode.DoubleRow

B, S, F, H, D = 2, 2048, 1024, 16, 64
NCORES = 8
RPC = 512            # query rows per core
CHUNKS = 4           # row chunks per batch (= cores per batch group)
KEYS = S             # 2048 keys per batch
NKT = KEYS // 128    # 16 key tiles
NPAIR = H // 2       # 8 head pairs
EPS = 1e-5

# (pair, kt) exp tiles offloaded to DVE via the int8 e4m3 bit-trick:
# value = bitcast_e4m3(round(score * 8*log2(e)/sqrt(D) + 55.54)), a
# piecewise-linear exp2 whose error (~3% rms) is comparable to the fp8
# quantization the ACT path already incurs
DVE_EXP = ({(p, kt) for p in range(1, 8) for kt in (3, 7, 11, 14)}
           - {(7, 14), (7, 11), (6, 14)}) | {(7, 2), (7, 5), (6, 5)}
EXP_A = 8.0 * 1.4426950408889634 / 8.0   # 8*log2e*inv_sqrt_d
EXP_B = 55.54

A1_J = 2                              # head pairs in the first A gather
A1_ELEMS = 64 * A1_J * 2 * RPC
A2_ELEMS = 64 * (8 - A1_J) * 2 * RPC
B_ELEMS = 128 * 4 * H * 65            # V own 4 keytiles [128, 4, 16, 65]

DEBUG = False
NO_COLL = False
NO_COLL_FREE = False
APPLY_GB = True


def _perm():
    """Weight-column permutation for the d-split score layout.
    Projection psum j covers head pair g=j: partitions 0:64 hold d-slot t=0
    (rows 32*hh+r for head h=2g+hh, d=r), partitions 64:128 hold slot t=1
    (d=32+r). new col 128*j + p <- old col 128*j + 64*hh + 32*t + r with
    hh=(p%64)//32, t=p//64, r=p%32."""
    p = np.empty(F, np.int64)
    for j in range(8):
        for pp in range(128):
            hh, t, r = (pp % 64) // 32, pp // 64, pp % 32
            p[128 * j + pp] = 128 * j + 64 * hh + 32 * t + r
    return p


PERM = _perm()


def _build_kernel():
    nc = bacc.Bacc(
        "TRN2",
        target_bir_lowering=False,
        debug=False,
        enable_asserts=False,
        num_devices=NCORES,
    )

    qT_d = nc.dram_tensor("qT", [F, RPC], FP8, kind="ExternalInput")
    qT16_d = nc.dram_tensor("qT16", [F, RPC], BF16, kind="ExternalInput")
    vT_d = nc.dram_tensor("vT", [F, RPC], FP8, kind="ExternalInput")
    wqp_d = nc.dram_tensor("wqp", [F, F], FP8, kind="ExternalInput")
    wvp_d = nc.dram_tensor("wvp", [F, F], FP8, kind="ExternalInput")
    wvn_d = nc.dram_tensor("wvn", [F, F], FP8, kind="ExternalInput")
    wfc_d = nc.dram_tensor("wfc", [2 * F, F], BF16, kind="ExternalInput")
    bqp_d = nc.dram_tensor("bqp", [128, 8], F32, kind="ExternalInput")
    bvp_d = nc.dram_tensor("bvp", [128, 8], F32, kind="ExternalInput")
    bvr_d = nc.dram_tensor("bvr", [1, F], F32, kind="ExternalInput")
    bfc_d = nc.dram_tensor("bfc", [1, F], BF16, kind="ExternalInput")
    gam_d = nc.dram_tensor("gam", [1, F], BF16, kind="ExternalInput")
    bet_d = nc.dram_tensor("bet", [1, F], BF16, kind="ExternalInput")
    out_d = nc.dram_tensor("out", [RPC, F], F32, kind="ExternalOutput")
    dbg = None
    if DEBUG:
        dbg = {
            "dbg_qTs": nc.dram_tensor("dbg_qTs", [64, 16 * RPC], FP8,
                                      kind="ExternalOutput"),
            "dbg_vTs": nc.dram_tensor("dbg_vTs", [64, 16 * KEYS], FP8,
                                      kind="ExternalOutput"),
            "dbg_V": nc.dram_tensor("dbg_V", [128, NKT * H * 65], FP8,
                                    kind="ExternalOutput"),
            "dbg_pt": nc.dram_tensor("dbg_pt", [128, NKT * 1024], FP8,
                                     kind="ExternalOutput"),
            "dbg_ctx": nc.dram_tensor("dbg_ctx", [128, NPAIR * RPC], BF16,
                                      kind="ExternalOutput"),
        }

    with tile.TileContext(nc) as tc:
        _kernel_body(tc, qT_d, qT16_d, vT_d, wqp_d, wvp_d, wvn_d, wfc_d,
                     bqp_d, bvp_d, bvr_d, bfc_d, gam_d, bet_d, out_d, dbg)

    nc.compile()
    return nc


def _stub_src(t, n):
    # [1, n] read of a tile's first row (dep-stub source)
    idx = (slice(0, 1),) + (0,) * (len(t.shape) - 2) + (slice(0, n),)
    return t[idx]


def _bcast_ap(t, nparts, n, row=0):
    # t: dram_tensor handle (has .ap() method) or an AP (dram pool tile)
    base = t.ap() if callable(getattr(t, "ap", None)) else t
    return bass.AP(tensor=base.tensor, offset=base.offset + row * n,
                   ap=[[0, nparts], [1, n]])


def _kernel_body(tc, qT_d, qT16_d, vT_d, wqp_d, wvp_d, wvn_d, wfc_d,
                 bqp_d, bvp_d, bvr_d, bfc_d, gam_d, bet_d, out_d, dbg=None):
    nc = tc.nc
    Exp = mybir.ActivationFunctionType.Exp
    Sqrt = mybir.ActivationFunctionType.Sqrt
    Ident = mybir.ActivationFunctionType.Identity
    Square = mybir.ActivationFunctionType.Square
    mult = mybir.AluOpType.mult
    addop = mybir.AluOpType.add

    import contextlib
    ctx = contextlib.ExitStack()
    with ctx:
        persist = ctx.enter_context(tc.tile_pool(name="persist", bufs=1))
        ptpool = ctx.enter_context(tc.tile_pool(name="ptpool", bufs=2))
        lnp = ctx.enter_context(tc.tile_pool(name="lnp", bufs=2))
        fcpool = ctx.enter_context(tc.tile_pool(name="fcpool", bufs=4))
        wblk = ctx.enter_context(tc.tile_pool(name="wblk", bufs=8))
        small = ctx.enter_context(tc.tile_pool(name="small", bufs=2))
        bcpool = ctx.enter_context(tc.tile_pool(name="bcpool", bufs=2))
        pps = ctx.enter_context(tc.tile_pool(name="pps", bufs=2, space="PSUM"))
        pctx = ctx.enter_context(tc.tile_pool(name="pctx", bufs=1, space="PSUM"))
        pfc = ctx.enter_context(tc.tile_pool(name="pfc", bufs=2, space="PSUM"))
        dram = ctx.enter_context(tc.tile_pool(name="dram", bufs=1, space="DRAM"))

        # ---- persistent SBUF ----
        qTin = persist.tile([128, 8, RPC], FP8)       # query chunk, dtile-major
        vTin = persist.tile([128, 8, RPC], FP8)       # value chunk
        qT_s = persist.tile([64, 8, 2, RPC], FP8)     # projected q, d-split layout
        # separate A1/A2 gather targets and staging: the A1 chain must not
        # wait on later pairs' projection stores, and the A2 scatter must not
        # carry a conservative WAR against pair-0/1 score reads
        vT_sA = persist.tile([64, A1_J, 2, KEYS], FP8)
        vT_sB = persist.tile([64, 8 - A1_J, 2, KEYS], FP8)
        V_full = persist.tile([128, NKT, H, 65], FP8) # projected V, 65-col head blocks
        vTstageA = persist.tile([64, A1_J, 2, RPC], FP8)
        vTstage = persist.tile([64, 8 - A1_J, 2, RPC], FP8)
        Vstage = persist.tile([128, 4, H, 65], FP8)   # own V keytiles (AG payload B)
        ctxT = persist.tile([128, NPAIR, RPC], BF16)  # normalized context^T
        qT16 = persist.tile([128, 8, RPC], BF16)      # query chunk bf16 (fc lhsT)
        fcq = persist.tile([128, 8, 512], F32)        # fc partial sums (qb, c)
        ones_bf = persist.tile([1, 512], BF16)
        ones64f = persist.tile([1, 64], BF16)
        bq_sb = persist.tile([128, 8], F32)
        bv_sb = persist.tile([128, 8], F32)
        bvr_bc = persist.tile([128, F], F32)
        bfc_sb = persist.tile([1, F], BF16)
        eps_sb = persist.tile([128, 1], F32)
        ssum_all = persist.tile([128, 8], F32)
        sqsum_all = persist.tile([128, 8], F32)
        if APPLY_GB:
            gamma_bc = persist.tile([128, F], BF16)
            beta_bc = persist.tile([128, F], BF16)

        ag_inA1 = dram.tile([A1_ELEMS], FP8)
        ag_outA1 = dram.tile([CHUNKS, A1_ELEMS], FP8)
        ag_inA2 = dram.tile([A2_ELEMS], FP8)
        ag_outA2 = dram.tile([CHUNKS, A2_ELEMS], FP8)
        ag_inB1 = dram.tile([B_ELEMS // 2], FP8)
        ag_outB1 = dram.tile([CHUNKS, B_ELEMS // 2], FP8)
        ag_inB2 = dram.tile([B_ELEMS // 2], FP8)
        ag_outB2 = dram.tile([CHUNKS, B_ELEMS // 2], FP8)
        rec_d = dram.tile([H, RPC], BF16)             # normalize recip bounce

        nc.vector.memset(ones_bf[:, :], 1.0)
        nc.vector.memset(ones64f[:, :], 1.0)
        nc.vector.memset(eps_sb[:, :], EPS)
        nc.vector.memset(Vstage[:, :, :, 64:65], 1.0)

        # ---- input DMAs: the vT path first (it feeds the A1 gather chain),
        # then the q path ----
        wvp_blks = []
        wqp_blks = []
        for kp in range(4):
            wb = wblk.tile([128, 2, F], FP8, tag="wblk", name=f"wvp{kp}")
            nc.sync.dma_start(
                out=wb[:, :, :],
                in_=wvp_d[256 * kp:256 * (kp + 1), :].rearrange(
                    "(t p) n -> p t n", p=128))
            wvp_blks.append(wb)
            nc.sync.dma_start(
                out=vTin[:, 2 * kp:2 * kp + 2, :],
                in_=vT_d[256 * kp:256 * (kp + 1), :].rearrange(
                    "(m p) n -> p m n", p=128))
        nc.sync.dma_start(out=bv_sb[:, :], in_=bvp_d[:, :])
        for kp in range(4):
            wq = wblk.tile([128, 2, F], FP8, tag="wblk", name=f"wqp{kp}")
            nc.sync.dma_start(
                out=wq[:, :, :],
                in_=wqp_d[256 * kp:256 * (kp + 1), :].rearrange(
                    "(t p) n -> p t n", p=128))
            wqp_blks.append(wq)
            nc.sync.dma_start(
                out=qTin[:, 2 * kp:2 * kp + 2, :],
                in_=qT_d[256 * kp:256 * (kp + 1), :].rearrange(
                    "(m p) n -> p m n", p=128))
        nc.sync.dma_start(out=bq_sb[:, :], in_=bqp_d[:, :])

        def all_gather(in_ap, out_ap):
            if NO_COLL_FREE:
                nc.sync.dma_start(out=out_ap[0], in_=in_ap)
            elif NO_COLL:
                n = in_ap.free_size() * in_ap.partition_size() \
                    if hasattr(in_ap, 'partition_size') else None
                src_bc = bass.AP(tensor=in_ap.tensor, offset=in_ap.offset,
                                 ap=[[0, CHUNKS]] + list(in_ap.ap))
                nc.sync.dma_start(out=out_ap[:, :], in_=src_bc)
            else:
                nc.gpsimd.collective_compute(
                    "AllGather",
                    mybir.AluOpType.bypass,
                    replica_groups=[[0, 1, 2, 3], [4, 5, 6, 7]],
                    ins=[in_ap],
                    outs=[out_ap],
                )

        def gather_chain_A(agin, agout, srctile, npairs, dsttile):
            nj = 2 * npairs
            nc.sync.dma_start(
                out=agin[:].rearrange("(j p n) -> p j n", p=64, j=nj),
                in_=srctile[0:64, :, :, :].rearrange("p g t n -> p (g t) n"))
            all_gather(agin[:], agout[:, :])
            for r in range(CHUNKS):
                nc.sync.dma_start(
                    out=dsttile[0:64, :, :, :].rearrange(
                        "p g t n -> p (g t) n")[:, :, r * RPC:(r + 1) * RPC],
                    in_=agout[r, :].rearrange("(j p n) -> p j n", p=64, j=nj))

        def proj_j(blks, xin, dst, dstj, bias_sb, ps_j, j, act_store=False):
            for kp in range(4):
                nc.tensor.matmul(ps_j[:, :],
                                 blks[kp][:, :, 128 * j:128 * (j + 1)],
                                 xin[:, 2 * kp:2 * kp + 2, :],
                                 start=(kp == 0), stop=(kp == 3),
                                 perf_mode=DR)
            with nc.allow_low_precision(reason="fp8 proj store"):
                if act_store:
                    # ACT is idle pre-attention; bias-add store via
                    # activation Identity shortens the serial DVE store
                    # chain that gates the A2 stage
                    nc.scalar.activation(
                        dst[0:64, dstj, 0, :], ps_j[0:64, :],
                        mybir.ActivationFunctionType.Identity,
                        bias=bias_sb[0:64, j:j + 1])
                else:
                    nc.vector.tensor_scalar(
                        dst[0:64, dstj, 0, :], ps_j[0:64, :],
                        bias_sb[0:64, j:j + 1], None, op0=addop)
                nc.vector.tensor_scalar(
                    dst[0:64, dstj, 1, :], ps_j[64:128, :],
                    bias_sb[64:128, j:j + 1], None, op0=addop)

        # ---- pre-attention: vT_s pairs 0,1 -> gather A1 -> q pairs 0,1 ----
        pre = []
        for i in range(2):
            big = pps.tile([128, 1024], F32, tag="ps", name=f"pre{i}")
            pre.append(big[:, 0:512])
            pre.append(big[:, 512:1024])
        for j in range(A1_J):
            proj_j(wvp_blks, vTin, vTstageA, j, bv_sb, pre[j], j,
                   act_store=True)
        gather_chain_A(ag_inA1, ag_outA1, vTstageA, A1_J, vT_sA)
        for j in range(2):
            proj_j(wqp_blks, qTin, qT_s, j, bq_sb, pre[2 + j], j)

        # ---- bulk DMAs deferred behind the A1 chain: tiny ACT copies
        # reading vT_s (written by the A1 scatters) gate them via WAW on
        # their destination tiles, keeping the DMA device clear while the
        # first scores' operands land ----
        Copy = mybir.ActivationFunctionType.Copy
        nc.scalar.activation(bvr_bc[0:1, 0:1], _stub_src(vT_sA, 1), Copy)
        nc.sync.dma_start(out=bvr_bc[:, :], in_=_bcast_ap(bvr_d, 128, F))
        wvn_blks = []
        for kp in range(4):
            wb = wblk.tile([128, 2, F], FP8, tag="wblk", name=f"wvn{kp}")
            nc.scalar.activation(wb[0:1, 0, 0:1], _stub_src(vT_sA, 1), Copy)
            nc.sync.dma_start(
                out=wb[:, :, :],
                in_=wvn_d[256 * kp:256 * (kp + 1), :].rearrange(
                    "(t p) n -> p t n", p=128))
            wvn_blks.append(wb)
        nc.scalar.activation(qT16[0:1, :, 0:1], _stub_src(vT_sB, 8), Copy)
        for m in range(8):
            nc.sync.dma_start(
                out=qT16[:, m, :],
                in_=qT16_d[128 * m:128 * (m + 1), :])
        wfc_blks = [None] * 8

        # ---- attention-phase work units (hooked into the kt loop) ----
        inv_sqrt_d = 1.0 / np.sqrt(D)
        pt_tiles = {}
        cps_tiles = {}

        def hook_psum(name):
            # rotate the 4 non-score psum tiles for hook-phase matmul groups
            i = hook_psum.n % 4
            hook_psum.n += 1
            if i < 2:
                return pctx.tile([128, RPC], F32, tag=("ctxA", "ctxB")[i],
                                 name=name)
            return pfc.tile([128, RPC], F32, tag="fc", name=name)
        hook_psum.n = 0

        def proj_hook(blks, xin, dst, dstj, bias_sb, j, act_store=False):
            proj_j(blks, xin, dst, dstj, bias_sb, hook_psum(f"pj{j}"), j,
                   act_store=act_store)

        def v_group(l, c):
            # V projection group: own keytile l, column half c
            ps_ = hook_psum(f"vg{l}{c}")
            for kp in range(4):
                nc.tensor.matmul(ps_[:, :],
                                 vTin[:, 2 * kp:2 * kp + 2,
                                      128 * l:128 * (l + 1)],
                                 wvn_blks[kp][:, :, 512 * c:512 * (c + 1)],
                                 start=(kp == 0), stop=(kp == 3),
                                 perf_mode=DR)
            with nc.allow_low_precision(reason="fp8 V store"):
                nc.vector.scalar_tensor_tensor(
                    Vstage[:, l, 8 * c:8 * (c + 1), 0:64],
                    ps_[:, :].rearrange("p (h d) -> p h d", d=64),
                    1.0,
                    bvr_bc[:, 512 * c:512 * (c + 1)].rearrange(
                        "p (h d) -> p h d", d=64),
                    op0=mult, op1=addop)

        def gather_chain_B(ls, agin, agout):
            # gather half the V keytiles (ls = (0,1) or (2,3))
            nl = len(ls)
            nc.sync.dma_start(
                out=agin[:].rearrange("(l p n) -> p l n", p=128, l=nl),
                in_=Vstage[:, ls[0]:ls[0] + nl, :, :].rearrange(
                    "p l h e -> p l (h e)"))
            all_gather(agin[:], agout[:, :])
            for r in range(CHUNKS):
                nc.sync.dma_start(
                    out=V_full[:, 4 * r + ls[0]:4 * r + ls[0] + nl, :, :]
                        .rearrange("p l h e -> p l (h e)"),
                    in_=agout[r, :].rearrange("(l p n) -> p l n", p=128, l=nl))

        def ctx_part(pp, j2s, first, last):
            if first:
                cps_tiles[pp] = (
                    pctx.tile([65, RPC], F32, tag="ctxA", name=f"cA{pp}"),
                    pctx.tile([65, RPC], F32, tag="ctxB", name=f"cB{pp}"))
            cpsA, cpsB = cps_tiles[pp]
            pt = pt_tiles[pp]
            for n, j2 in enumerate(j2s):
                for hh, cps in ((0, cpsA), (1, cpsB)):
                    nc.tensor.matmul(
                        cps[:, :],
                        V_full[:, 2 * j2:2 * j2 + 2, 2 * pp + hh, :],
                        pt[:, 2 * j2:2 * j2 + 2, hh, :],
                        start=(first and n == 0),
                        stop=(last and n == len(j2s) - 1),
                        perf_mode=DR)

        def ctx_burst(pp):
            # B1 keytiles (4r, 4r+1 -> even j2) first, then B2 (odd j2)
            ctx_part(pp, [0, 2, 4, 6, 1, 3, 5, 7], True, True)

        def normalize_pair(pp, pe_bc=False):
            cpsA, cpsB = cps_tiles[pp]
            if pe_bc:
                # tail path: hoist both recips, do the psum->SBUF broadcast
                # copies on ACT (idle after the last exp) so DVE only runs
                # recipA, recipB, multA, multB on the critical chain
                Copy_ = mybir.ActivationFunctionType.Copy
                recs, bcss = [], []
                for hh, cps in ((0, cpsA), (1, cpsB)):
                    rec = small.tile([1, RPC], BF16, tag="rec", name="rec")
                    with nc.allow_low_precision(
                            reason="softmax denom recip bf16"):
                        nc.vector.reciprocal(rec[:, :], cps[64:65, :])
                    recs.append(rec)
                for hh, cps in ((0, cpsA), (1, cpsB)):
                    bcs = bcpool.tile([64, RPC], BF16, tag="bcs", name="bcs")
                    bcp = pfc.tile([64, RPC], F32, tag="fc", name="bcp")
                    nc.tensor.matmul(bcp[:, :], ones64f[:, :],
                                     recs[hh][:, :], start=True, stop=True)
                    with nc.allow_low_precision(reason="bcs copy bf16"):
                        nc.scalar.activation(bcs[:, :], bcp[:, :], Copy_)
                    bcss.append(bcs)
                for hh, cps in ((0, cpsA), (1, cpsB)):
                    nc.vector.tensor_tensor(
                        ctxT[64 * hh:64 * (hh + 1), pp, :],
                        cps[0:64, :], bcss[hh][:, :], op=mult)
                return
            for hh, cps in ((0, cpsA), (1, cpsB)):
                rec = small.tile([1, RPC], BF16, tag="rec", name="rec")
                with nc.allow_low_precision(reason="softmax denom recip bf16"):
                    nc.vector.reciprocal(rec[:, :], cps[64:65, :])
                bcs = bcpool.tile([64, RPC], BF16, tag="bcs", name="bcs")
                nc.sync.dma_start(out=rec_d[2 * pp + hh, :], in_=rec[:, :])
                nc.sync.dma_start(
                    out=bcs[:, :],
                    in_=_bcast_ap(rec_d, 64, RPC, row=2 * pp + hh))
                nc.vector.tensor_tensor(
                    ctxT[64 * hh:64 * (hh + 1), pp, :],
                    cps[0:64, :], bcs[:, :], op=mult)

        def fc_session(i, kcs, first=False, last=False):
            qb, c = i // 2, i % 2
            ps_ = pfc.tile([128, RPC], F32, tag="fc", name=f"s{i}")
            kcs = list(kcs)
            for n, kc in enumerate(kcs):
                if kc < 8:
                    lhsT = ctxT[:, kc, 128 * qb:128 * (qb + 1)]
                else:
                    lhsT = qT16[:, kc - 8, 128 * qb:128 * (qb + 1)]
                nc.tensor.matmul(
                    ps_[:, :], lhsT,
                    wfc_blks[kc // 2][:, kc % 2, 512 * c:512 * (c + 1)],
                    start=(n == 0),
                    stop=(not first and n == len(kcs) - 1))
            if first:
                # fold the fc bias into the first session
                nc.tensor.matmul(ps_[:, :], ones_bf[:, 0:128],
                                 bfc_sb[:, 512 * c:512 * (c + 1)],
                                 start=False, stop=True)
                nc.vector.tensor_copy(fcq[:, i, :], ps_[:, :])
            elif last:
                nc.vector.scalar_tensor_tensor(
                    fcq[:, i, :], ps_[:, :], 1.0, fcq[:, i, :],
                    op0=mult, op1=addop, accum_out=ssum_all[:, i:i + 1])
            else:
                nc.vector.tensor_tensor(fcq[:, i, :], ps_[:, :], fcq[:, i, :],
                                        op=addop)

        # hook schedule: (pair, kt) -> list of work closures
        hooks = {}

        def add_hook(p, kt, fn):
            hooks.setdefault((p, kt), []).append(fn)

        # pair 0: finish vT pairs 1-7, gather A2, V groups + split B gathers
        for j in range(A1_J, 8):
            add_hook(0, j - A1_J, lambda j=j: proj_hook(wvp_blks, vTin,
                                                        vTstage, j - A1_J,
                                                        bv_sb, j,
                                                        act_store=True))
        add_hook(0, 6, lambda: gather_chain_A(ag_inA2, ag_outA2, vTstage,
                                              8 - A1_J, vT_sB))
        for g in range(4):
            add_hook(0, 8 + g, lambda g=g: v_group(g // 2, g % 2))
        add_hook(0, 11, lambda: gather_chain_B((0, 1), ag_inB1, ag_outB1))
        for g in range(4, 8):
            add_hook(0, 8 + g, lambda g=g: v_group(g // 2, g % 2))
        def late_dmas():
            with nc.allow_low_precision(reason="dep stub"):
                nc.gpsimd.tensor_copy(bfc_sb[0:1, 0:1].bitcast(FP8)[:, 0:1],
                                      V_full[0:1, 13, 0, 0:1])
            nc.sync.dma_start(out=bfc_sb[:, :], in_=bfc_d[:, :])
            if APPLY_GB:
                nc.sync.dma_start(out=gamma_bc[:, :],
                                  in_=_bcast_ap(gam_d, 128, F))
                nc.sync.dma_start(out=beta_bc[:, :],
                                  in_=_bcast_ap(bet_d, 128, F))
            for jj in (4, 5, 6, 7, 0, 1, 2, 3):
                wb = wblk.tile([128, 2, F], BF16, tag="wblk", name=f"wfc{jj}")
                nc.sync.dma_start(
                    out=wb[:, :, :],
                    in_=wfc_d[256 * jj:256 * (jj + 1), :].rearrange(
                        "(t p) n -> p t n", p=128))
                wfc_blks[jj] = wb

        # pair 1: B2 gather, late bulk DMAs, q pairs 2-7
        add_hook(1, 0, lambda: gather_chain_B((2, 3), ag_inB2, ag_outB2))
        add_hook(1, 1, late_dmas)
        for j in range(2, 8):
            add_hook(1, j - 1, lambda j=j: proj_hook(wqp_blks, qTin, qT_s, j,
                                                     bq_sb, j))
        if dbg is not None:
            def dump_pt0():
                nc.sync.dma_start(
                    out=dbg["dbg_pt"][:, :],
                    in_=pt_tiles[0][:, :, :, :].rearrange("p k h n -> p (k h n)"))
            add_hook(1, 7, dump_pt0)
        # pair 1 tail/pair 2: ctx/norm pair 0, ctx pair 1
        add_hook(1, 12, lambda: ctx_burst(0))
        add_hook(2, 4, lambda: normalize_pair(0))
        add_hook(2, 8, lambda: ctx_burst(1))
        # pair 3: norm 1, ctx/norm 2, fc query sessions 0-3
        add_hook(2, 15, lambda: normalize_pair(1))
        for i in range(4):
            add_hook(3, 2 + 4 * i, lambda i=i: fc_session(i, range(8, 16),
                                                          first=True))
        add_hook(3, 6, lambda: ctx_burst(2))
        add_hook(3, 12, lambda: normalize_pair(2))
        # pair 4: ctx/norm 3, fc query sessions 4-7
        for i in range(4):
            add_hook(4, 2 + 4 * i, lambda i=i: fc_session(4 + i, range(8, 16),
                                                          first=True))
        add_hook(4, 4, lambda: ctx_burst(3))
        add_hook(4, 12, lambda: normalize_pair(3))
        for p in (5, 6):
            add_hook(p, 2, lambda pp=p - 1: ctx_burst(pp))
            add_hook(p, 10, lambda pp=p - 1: normalize_pair(pp))
        for i in range(4):
            add_hook(5, 2 + 4 * i, lambda i=i: fc_session(i, range(0, 4)))
            add_hook(6, 2 + 4 * i, lambda i=i: fc_session(4 + i, range(0, 4)))
        # pair 7: ctx/norm 6 early, fc kc 4-6 sessions, ctx 7 nearly inline
        add_hook(7, 0, lambda: ctx_burst(6))
        add_hook(7, 2, lambda: normalize_pair(6))
        for i in range(8):
            add_hook(7, 8 + i, lambda i=i: fc_session(i, range(4, 7)))
        add_hook(7, 14, lambda: ctx_part(7, [0, 2, 4, 1, 3, 5, 6], True, False))

        # ---- attention ----
        for p in range(NPAIR):
            pt = ptpool.tile([128, NKT, 2, RPC], FP8, tag="pt", name="pt")
            pt_tiles[p] = pt
            vts, vj = (vT_sA, p) if p < A1_J else (vT_sB, p - A1_J)
            for kt in range(NKT):
                if (p, kt) in DVE_EXP:
                    # DVE-exp tiles use the fc psum ring so the ACT stream's
                    # pps ring stays an uninterrupted double-buffer
                    with nc.allow_low_precision(reason="dve int8 exp trick"):
                        for hh in range(2):
                            psd = pfc.tile([128, RPC], F32, tag="fc",
                                           name=f"dve{hh}")
                            nc.tensor.matmul(
                                psd[:, :],
                                vts[32 * hh:32 * (hh + 1), vj, :,
                                    128 * kt:128 * (kt + 1)],
                                qT_s[32 * hh:32 * (hh + 1), p, :, :],
                                start=True, stop=True, perf_mode=DR)
                            nc.vector.tensor_scalar(
                                pt[:, kt, hh, :].bitcast(I8), psd[:, :],
                                EXP_A, EXP_B, op0=mult, op1=addop)
                else:
                    ps = pps.tile([128, 1024], F32, tag="ps", name="ps")
                    for hh in range(2):
                        nc.tensor.matmul(
                            ps[:, 512 * hh:512 * (hh + 1)],
                            vts[32 * hh:32 * (hh + 1), vj, :,
                                128 * kt:128 * (kt + 1)],
                            qT_s[32 * hh:32 * (hh + 1), p, :, :],
                            start=True, stop=True, perf_mode=DR)
                    nc.scalar.activation(pt[:, kt, :, :], ps[:, :], Exp,
                                         scale=inv_sqrt_d)
                for fn in hooks.get((p, kt), ()):
                    fn()

        # ---- tail: finish ctx 7, normalize (PE broadcast), final fc, LN ----
        # dummy Sqrt preloads the sqrt table set while ACT is otherwise idle
        dmy = small.tile([1, 1], F32, tag="dmy", name="dmy")
        nc.scalar.activation(dmy[0:1, 0:1], eps_sb[0:1, 0:1], Sqrt)
        ctx_part(7, [7], False, True)
        normalize_pair(7, pe_bc=True)
        if dbg is not None:
            nc.sync.dma_start(out=dbg["dbg_qTs"][:, :],
                              in_=qT_s[:, :, :, :].rearrange("p g t n -> p (g t n)"))
            nc.sync.dma_start(out=dbg["dbg_vTs"][:, :],
                              in_=vT_sB[:, :, :, :].rearrange("p g t n -> p (g t n)"))
            nc.sync.dma_start(out=dbg["dbg_V"][:, :],
                              in_=V_full[:, :, :, :].rearrange("p k h e -> p (k h e)"))
            nc.sync.dma_start(out=dbg["dbg_ctx"][:, :],
                              in_=ctxT[:, :, :].rearrange("p q n -> p (q n)"))
        for i in range(8):
            fc_session(i, range(7, 8), last=True)

        # LayerNorm: sq-stats on ACT (sum came free from the fc adds), the
        # per-row scalar chain batched into [128, 4] columns, affines split
        # across ACT/DVE.
        for i in range(8):
            dump = lnp.tile([128, 512], F32, tag=f"sq{i % 4}", name="dump",
                            bufs=1)
            eng = (nc.scalar, nc.scalar, nc.scalar, nc.vector)[i % 4]
            if eng is nc.scalar:
                nc.scalar.activation(dump[:, :], fcq[:, i, :], Square,
                                     accum_out=sqsum_all[:, i:i + 1])
            else:
                eng.scalar_tensor_tensor(
                    dump[:, :], fcq[:, i, :], 1.0, fcq[:, i, :],
                    op0=mult, op1=mult, accum_out=sqsum_all[:, i:i + 1])
        # stats split into two per-half chains so the first row blocks'
        # affines and output DMAs start before the last squares finish
        rstds, nmrs = [], []
        for hf in range(2):
            c0 = 4 * hf
            mean = small.tile([128, 2], F32, tag=f"mean{hf}", name="mean")
            nc.vector.tensor_tensor(mean[:, :], ssum_all[:, c0:c0 + 4:2],
                                    ssum_all[:, c0 + 1:c0 + 4:2], op=addop)
            nc.vector.tensor_scalar(mean[:, :], mean[:, :], 1.0 / F, None,
                                    op0=mult)
            ex2 = small.tile([128, 2], F32, tag=f"ex2{hf}", name="ex2")
            nc.vector.tensor_tensor(ex2[:, :], sqsum_all[:, c0:c0 + 4:2],
                                    sqsum_all[:, c0 + 1:c0 + 4:2], op=addop)
            nc.vector.tensor_scalar(ex2[:, :], ex2[:, :], 1.0 / F, None,
                                    op0=mult)
            var = small.tile([128, 2], F32, tag=f"var{hf}", name="var")
            nc.vector.scalar_tensor_tensor(var[:, :], mean[:, :], -1.0,
                                           mean[:, :], op0=mult, op1=mult)
            nc.vector.tensor_tensor(var[:, :], ex2[:, :], var[:, :], op=addop)
            sd = small.tile([128, 2], F32, tag=f"sd{hf}", name="sd")
            nc.scalar.activation(sd[:, :], var[:, :], Sqrt, bias=eps_sb[:, :])
            rstd = small.tile([128, 2], F32, tag=f"rstd{hf}", name="rstd")
            nc.vector.reciprocal(rstd[:, :], sd[:, :])
            nmr = small.tile([128, 2], F32, tag=f"nmr{hf}", name="nmr")
            nc.vector.scalar_tensor_tensor(nmr[:, :], mean[:, :], -1.0,
                                           rstd[:, :], op0=mult, op1=mult)
            rstds.append(rstd)
            nmrs.append(nmr)
        for i in range(8):
            qb, c = i // 2, i % 2
            sl = slice(512 * c, 512 * (c + 1))
            outt = fcpool.tile([128, 512], F32, tag="outt", name="outt")
            hf, qh = qb // 2, qb % 2
            if i % 2 == 0:
                nc.scalar.activation(outt[:, :], fcq[:, i, :], Ident,
                                     bias=nmrs[hf][:, qh:qh + 1],
                                     scale=rstds[hf][:, qh:qh + 1])
            else:
                nc.vector.tensor_scalar(outt[:, :], fcq[:, i, :],
                                        rstds[hf][:, qh:qh + 1],
                                        nmrs[hf][:, qh:qh + 1],
                                        op0=mult, op1=addop)
            if APPLY_GB:
                t2 = lnp.tile([128, 512], F32, tag="t1", name="t2",
                              bufs=1)
                nc.vector.tensor_tensor(t2[:, :], outt[:, :],
                                        gamma_bc[:, sl], op=mult)
                nc.vector.tensor_tensor(outt[:, :], t2[:, :],
                                        beta_bc[:, sl], op=addop)
            nc.sync.dma_start(out=out_d[128 * qb:128 * (qb + 1), sl],
                              in_=outt[:, :])


_NC_CACHE = {}


def _get_nc():
    key = (APPLY_GB, NO_COLL, DEBUG)
    if key not in _NC_CACHE:
        _NC_CACHE[key] = _build_kernel()
    return _NC_CACHE[key]


def _prep_inputs(query, value, Wq, bq, Wv, bv, Wfc, bfc, gamma, beta):
    wqp = np.ascontiguousarray(Wq[:, PERM]).astype(NP_FP8)
    wvp = np.ascontiguousarray(Wv[:, PERM]).astype(NP_FP8)
    wvn = np.ascontiguousarray(Wv).astype(NP_FP8)
    wfc16 = np.ascontiguousarray(Wfc).astype(NP_BF16)
    bqp = np.ascontiguousarray(bq[PERM].reshape(8, 128).T).astype(np.float32)
    bvp = np.ascontiguousarray(bv[PERM].reshape(8, 128).T).astype(np.float32)
    bvr = np.ascontiguousarray(bv[None, :]).astype(np.float32)
    bfc16 = np.ascontiguousarray(bfc[None, :]).astype(NP_BF16)
    gam = np.ascontiguousarray(gamma[None, :]).astype(NP_BF16)
    bet = np.ascontiguousarray(beta[None, :]).astype(NP_BF16)

    in_maps = []
    for c in range(NCORES):
        b, r = c // CHUNKS, (c % CHUNKS) * RPC
        qTf = np.ascontiguousarray(query[b, r:r + RPC, :].T)
        qT = qTf.astype(NP_FP8)
        qT16 = qTf.astype(NP_BF16)
        vT = np.ascontiguousarray(value[b, r:r + RPC, :].T).astype(NP_FP8)
        in_maps.append({
            "qT": qT, "qT16": qT16, "vT": vT,
            "wqp": wqp, "wvp": wvp, "wvn": wvn, "wfc": wfc16,
            "bqp": bqp, "bvp": bvp, "bvr": bvr, "bfc": bfc16,
            "gam": gam, "bet": bet,
        })
    return in_maps


def run_on_hw(in_maps, **kwargs):
    nc = _get_nc()
    return run_bass_kernel_spmd(nc, in_maps, list(range(NCORES)), **kwargs)


def kernel(query, value, Wq, bq, Wv, bv, Wfc, bfc, gamma, beta):
    global APPLY_GB
    APPLY_GB = not (np.all(np.asarray(gamma, np.float32) == 1.0)
                    and np.all(np.asarray(beta, np.float32) == 0.0))
    query = np.asarray(query, dtype=np.float32)
    value = np.asarray(value, dtype=np.float32)
    in_maps = _prep_inputs(query, value,
                           np.asarray(Wq, np.float32), np.asarray(bq, np.float32),
                           np.asarray(Wv, np.float32), np.asarray(bv, np.float32),
                           np.asarray(Wfc, np.float32), np.asarray(bfc, np.float32),
                           np.asarray(gamma, np.float32), np.asarray(beta, np.float32))
    res = run_on_hw(in_maps)
    out = np.empty((B, S, F), np.float32)
    for c in range(NCORES):
        b, r = c // CHUNKS, (c % CHUNKS) * RPC
        out[b, r:r + RPC, :] = res.results[c]["out"]
    return out

